# revision 6
# baseline (speedup 1.0000x reference)
"""Cost-volume kernel for Trainium2 (Bass/Tile), 8-core SPMD.

Problem: left/right features [B=2, C=32, H=128, W=256] f32.
Output [B, 2C=64, D=48, H, W] where for disparity d in [-8, 40):
  out[:, 0:C,  d+8, h, x] = left[:, :, h, x]   if 0 <= x-d < W else 0
  out[:, C:2C, d+8, h, x] = right[:, :, h, x-d] if 0 <= x-d < W else 0

Sharding: channels split 4-per-core (8 cores, identical program).
Each core builds the full disparity band for its 4 left + 4 right
channels. Pure data-movement kernel, bound by the HBM write rate of
the 96 MiB/core output.

Perf notes (from NTFF traces):
  - HWDGE (sync/scalar) DMA rings only engage 8 of the 16 SDMA
    engines; SWDGE (gpsimd) engages all 16. All big transfers go SWDGE.
  - Every store is a full-width DMA with contiguous 8 KiB/partition
    source rows (128 descriptors of 8 KiB), which sustains near line
    rate. Right-side shifted windows are materialized by DVE copies
    into contiguous staging buffers to keep descriptors at 8 KiB.
  - Zero padding is produced in SBUF (host-padded right image, SBUF
    memsets for left), never as thin strided DRAM writes.
  - The right input arrives host-padded so no SBUF memset gates the
    first staging copies; left-buffer prep is emitted lazily to keep
    the gpsimd DMA FIFO from head-of-line blocking at startup.
"""

import numpy as np

B, C, H, W = 2, 32, 128, 256
MIN_D, MAX_D = -8, 40
D = MAX_D - MIN_D  # 48
N_CORES = 8
CPC = C // N_CORES  # 4 channels of each image per core
BC = B * CPC  # 8 (b, c) pairs per core

PAD_L = 39  # covers max shift d=39 (offset = x - d + PAD_L >= 0)
PAD_R = 9   # covers min shift d=-8 (x - d <= 263 -> offset 302 < 304)
WP = PAD_L + W + PAD_R  # 304

HL = 8            # h rows held per partition
HH = H // HL      # 16
NPART = BC * HH   # 128 partitions: p = (b*CPC + c)*HH + h_hi

POS_BUFS = 5  # left work buffers for d >= 0 (buffer j: d = j, j+5, ... asc)
NEG_BUFS = 2  # left work buffers for d < 0 (buffer j: d = -(j+1), -(j+1)-2, ... desc)
STAGE_BUFS = 12  # right staging rotation depth (deep: keeps SDMA queues fed)

# store order for the left side: negatives interleaved early; within a
# buffer positives ascend and negatives descend (zero regions only grow).
LEFT_ORDER = [0, -1, 1, -2, 2, 3, -3, 4, 5, -4, 6, 7, -5, 8, 9, -6, 10,
              11, -7, 12, 13, -8] + list(range(14, MAX_D))
assert sorted(LEFT_ORDER) == list(range(MIN_D, MAX_D))

_CACHE = {}


def _build_nc():
    import concourse.bacc as bacc
    import concourse.tile as tile
    import concourse.mybir as mybir

    f32 = mybir.dt.float32
    nc = bacc.Bacc(
        "TRN2",
        target_bir_lowering=False,
        debug=False,
        enable_asserts=False,
        num_devices=N_CORES,
    )
    left_in = nc.dram_tensor("left_in", [B, CPC, H, W], f32, kind="ExternalInput")
    right_in = nc.dram_tensor(
        "right_in", [B, CPC, H, WP], f32, kind="ExternalInput"
    )  # host-padded with zeros: data columns at [PAD_L, PAD_L + W)
    left_out = nc.dram_tensor(
        "left_out", [B, CPC, D, H, W], f32, kind="ExternalOutput"
    )
    right_out = nc.dram_tensor(
        "right_out", [B, CPC, D, H, W], f32, kind="ExternalOutput"
    )

    with tile.TileContext(nc) as tc:
        with (
            tc.tile_pool(name="pool", bufs=1) as pool,
            tc.tile_pool(name="stpool", bufs=STAGE_BUFS) as stpool,
        ):
            # ---- right image (pre-padded), loaded once ----
            rp = pool.tile([NPART, HL * WP], f32, tag="rp")
            rp3 = rp[:].rearrange("p (h w) -> p h w", h=HL)
            nc.gpsimd.dma_start(rp[:], right_in.ap())

            # ---- left work buffers; pos[0] is the load target, the rest
            # are DVE-copied lazily on first use ----
            pos = []
            neg = []
            for j in range(POS_BUFS):
                t = pool.tile([NPART, HL * W], f32, tag=f"lp{j}")
                pos.append([t, t[:].rearrange("p (h w) -> p h w", h=HL), False])
            for j in range(NEG_BUFS):
                t = pool.tile([NPART, HL * W], f32, tag=f"ln{j}")
                neg.append([t, t[:].rearrange("p (h w) -> p h w", h=HL), False])
            pos[0][2] = True  # loaded directly, no copy needed
            nc.gpsimd.dma_start(pos[0][0][:], left_in.ap())

            def emit_left(d):
                if d >= 0:
                    buf = pos[d % POS_BUFS]
                    t, t3, ready = buf
                    if not ready:
                        nc.scalar.copy(t[:], pos[0][0][:])
                        if d > 0:
                            nc.vector.memset(t3[:, :, 0:d], 0.0)
                        buf[2] = True
                    elif d >= POS_BUFS:
                        nc.vector.memset(t3[:, :, d - POS_BUFS : d], 0.0)
                else:
                    buf = neg[(-d - 1) % NEG_BUFS]
                    t, t3, ready = buf
                    if not ready:
                        nc.scalar.copy(t[:], pos[0][0][:])
                        nc.vector.memset(t3[:, :, W + d : W], 0.0)
                        buf[2] = True
                    else:
                        nc.vector.memset(t3[:, :, W + d : W + d + NEG_BUFS], 0.0)
                nc.gpsimd.dma_start(left_out.ap()[:, :, d - MIN_D, :, :], t[:])

            def emit_right(di):
                d = di + MIN_D
                a = PAD_L - d
                stage = stpool.tile([NPART, HL * W], f32, tag="st")
                st3 = stage[:].rearrange("p (h w) -> p h w", h=HL)
                # alternate DVE/ACT so neither engine's SBUF ports throttle
                # the SDMA engines' SBUF reads
                if di % 2 == 0:
                    nc.vector.tensor_copy(st3[:], rp3[:, :, a : a + W])
                else:
                    nc.scalar.copy(st3[:], rp3[:, :, a : a + W])
                nc.gpsimd.dma_start(right_out.ap()[:, :, di, :, :], stage[:])

            for step in range(D):
                emit_right(step)
                emit_left(LEFT_ORDER[step])

    nc.compile()
    return nc


def _get_nc():
    if "nc" not in _CACHE:
        _CACHE["nc"] = _build_nc()
    return _CACHE["nc"]


def kernel(left_feat, right_feat):
    from concourse.bass_utils import run_bass_kernel_spmd

    left = np.ascontiguousarray(np.asarray(left_feat), dtype=np.float32)
    right = np.ascontiguousarray(np.asarray(right_feat), dtype=np.float32)
    assert left.shape == (B, C, H, W) and right.shape == (B, C, H, W)

    nc = _get_nc()
    right_pad = np.zeros((B, C, H, WP), dtype=np.float32)
    right_pad[:, :, :, PAD_L : PAD_L + W] = right
    in_maps = []
    for m in range(N_CORES):
        sl = slice(m * CPC, (m + 1) * CPC)
        in_maps.append(
            {
                "left_in": np.ascontiguousarray(left[:, sl]),
                "right_in": np.ascontiguousarray(right_pad[:, sl]),
            }
        )
    res = run_bass_kernel_spmd(nc, in_maps, core_ids=list(range(N_CORES))).results

    out = np.empty((B, 2 * C, D, H, W), dtype=np.float32)
    for m in range(N_CORES):
        sl = slice(m * CPC, (m + 1) * CPC)
        out[:, sl] = res[m]["left_out"]
        out[:, C + m * CPC : C + (m + 1) * CPC] = res[m]["right_out"]
    return out


# revision 8
# speedup vs baseline: 1.0442x; 1.0442x over previous
"""Cost-volume kernel for Trainium2 (Bass/Tile), 8-core SPMD.

Problem: left/right features [B=2, C=32, H=128, W=256] f32.
Output [B, 2C=64, D=48, H, W] where for disparity d in [-8, 40):
  out[:, 0:C,  d+8, h, x] = left[:, :, h, x]   if 0 <= x-d < W else 0
  out[:, C:2C, d+8, h, x] = right[:, :, h, x-d] if 0 <= x-d < W else 0

Sharding: channels split 4-per-core (8 cores, identical program).
Each core builds the full disparity band for its 4 left + 4 right
channels. Pure data-movement kernel, bound by the HBM write rate of
the 96 MiB/core output.

Perf notes (from NTFF traces):
  - HWDGE (sync/scalar) DMA rings only engage 8 of the 16 SDMA
    engines; SWDGE (gpsimd) engages all 16. All big transfers go SWDGE.
  - Every store is a full-width DMA with contiguous 8 KiB/partition
    source rows (128 descriptors of 8 KiB), which sustains near line
    rate. Right-side shifted windows are materialized by DVE copies
    into contiguous staging buffers to keep descriptors at 8 KiB.
  - Zero padding is produced in SBUF (host-padded right image, SBUF
    memsets for left), never as thin strided DRAM writes.
  - The right input arrives host-padded so no SBUF memset gates the
    first staging copies; left-buffer prep is emitted lazily to keep
    the gpsimd DMA FIFO from head-of-line blocking at startup.
"""

import numpy as np

B, C, H, W = 2, 32, 128, 256
MIN_D, MAX_D = -8, 40
D = MAX_D - MIN_D  # 48
N_CORES = 8
CPC = C // N_CORES  # 4 channels of each image per core
BC = B * CPC  # 8 (b, c) pairs per core

PAD_L = 39  # covers max shift d=39 (offset = x - d + PAD_L >= 0)
PAD_R = 9   # covers min shift d=-8 (x - d <= 263 -> offset 302 < 304)
WP = PAD_L + W + PAD_R  # 304

HL = 8            # h rows held per partition
HH = H // HL      # 16
NPART = BC * HH   # 128 partitions: p = (b*CPC + c)*HH + h_hi

POS_BUFS = 4  # left work buffers for d >= 0 (buffer j: d = j, j+4, ... asc)
NEG_BUFS = 2  # left work buffers for d < 0 (buffer j: d = -(j+1), -(j+1)-2, ... desc)
STAGE_BUFS = 16  # right staging rotation depth (deep: keeps SDMA queues fed)

# store order for the left side: negatives interleaved early; within a
# buffer positives ascend and negatives descend (zero regions only grow).
LEFT_ORDER = [0, -1, 1, -2, 2, 3, -3, 4, 5, -4, 6, 7, -5, 8, 9, -6, 10,
              11, -7, 12, 13, -8] + list(range(14, MAX_D))
assert sorted(LEFT_ORDER) == list(range(MIN_D, MAX_D))

_CACHE = {}


def _build_nc():
    import concourse.bacc as bacc
    import concourse.tile as tile
    import concourse.mybir as mybir

    f32 = mybir.dt.float32
    nc = bacc.Bacc(
        "TRN2",
        target_bir_lowering=False,
        debug=False,
        enable_asserts=False,
        num_devices=N_CORES,
    )
    left_in = nc.dram_tensor("left_in", [B, CPC, H, W], f32, kind="ExternalInput")
    right_in = nc.dram_tensor(
        "right_in", [B, CPC, H, WP], f32, kind="ExternalInput"
    )  # host-padded with zeros: data columns at [PAD_L, PAD_L + W)
    left_out = nc.dram_tensor(
        "left_out", [B, CPC, D, H, W], f32, kind="ExternalOutput"
    )
    right_out = nc.dram_tensor(
        "right_out", [B, CPC, D, H, W], f32, kind="ExternalOutput"
    )

    with tile.TileContext(nc) as tc:
        with (
            tc.tile_pool(name="pool", bufs=1) as pool,
            tc.tile_pool(name="stpool", bufs=STAGE_BUFS) as stpool,
        ):
            # ---- right image (pre-padded), loaded once ----
            rp = pool.tile([NPART, HL * WP], f32, tag="rp")
            rp3 = rp[:].rearrange("p (h w) -> p h w", h=HL)
            # HWDGE (SP) load: lands while the Q7/SWDGE path is still
            # warming up, so the first staging copies start earlier
            nc.sync.dma_start(rp[:], right_in.ap())

            # ---- left work buffers; pos[0] is the load target, the rest
            # are DVE-copied lazily on first use ----
            pos = []
            neg = []
            for j in range(POS_BUFS):
                t = pool.tile([NPART, HL * W], f32, tag=f"lp{j}")
                pos.append([t, t[:].rearrange("p (h w) -> p h w", h=HL), False])
            for j in range(NEG_BUFS):
                t = pool.tile([NPART, HL * W], f32, tag=f"ln{j}")
                neg.append([t, t[:].rearrange("p (h w) -> p h w", h=HL), False])
            pos[0][2] = True  # loaded directly, no copy needed
            nc.gpsimd.dma_start(pos[0][0][:], left_in.ap())

            def emit_left(d):
                if d >= 0:
                    buf = pos[d % POS_BUFS]
                    t, t3, ready = buf
                    if not ready:
                        nc.scalar.copy(t[:], pos[0][0][:])
                        if d > 0:
                            nc.vector.memset(t3[:, :, 0:d], 0.0)
                        buf[2] = True
                    elif d >= POS_BUFS:
                        nc.vector.memset(t3[:, :, d - POS_BUFS : d], 0.0)
                else:
                    buf = neg[(-d - 1) % NEG_BUFS]
                    t, t3, ready = buf
                    if not ready:
                        nc.scalar.copy(t[:], pos[0][0][:])
                        nc.vector.memset(t3[:, :, W + d : W], 0.0)
                        buf[2] = True
                    else:
                        nc.vector.memset(t3[:, :, W + d : W + d + NEG_BUFS], 0.0)
                nc.gpsimd.dma_start(left_out.ap()[:, :, d - MIN_D, :, :], t[:])

            def emit_right(di):
                d = di + MIN_D
                a = PAD_L - d
                stage = stpool.tile([NPART, HL * W], f32, tag="st")
                st3 = stage[:].rearrange("p (h w) -> p h w", h=HL)
                nc.vector.tensor_copy(st3[:], rp3[:, :, a : a + W])
                nc.gpsimd.dma_start(right_out.ap()[:, :, di, :, :], stage[:])

            for step in range(D):
                emit_right(step)
                emit_left(LEFT_ORDER[step])

    nc.compile()
    return nc


def _get_nc():
    if "nc" not in _CACHE:
        _CACHE["nc"] = _build_nc()
    return _CACHE["nc"]


def kernel(left_feat, right_feat):
    from concourse.bass_utils import run_bass_kernel_spmd

    left = np.ascontiguousarray(np.asarray(left_feat), dtype=np.float32)
    right = np.ascontiguousarray(np.asarray(right_feat), dtype=np.float32)
    assert left.shape == (B, C, H, W) and right.shape == (B, C, H, W)

    nc = _get_nc()
    right_pad = np.zeros((B, C, H, WP), dtype=np.float32)
    right_pad[:, :, :, PAD_L : PAD_L + W] = right
    in_maps = []
    for m in range(N_CORES):
        sl = slice(m * CPC, (m + 1) * CPC)
        in_maps.append(
            {
                "left_in": np.ascontiguousarray(left[:, sl]),
                "right_in": np.ascontiguousarray(right_pad[:, sl]),
            }
        )
    res = run_bass_kernel_spmd(nc, in_maps, core_ids=list(range(N_CORES))).results

    out = np.empty((B, 2 * C, D, H, W), dtype=np.float32)
    for m in range(N_CORES):
        sl = slice(m * CPC, (m + 1) * CPC)
        out[:, sl] = res[m]["left_out"]
        out[:, C + m * CPC : C + (m + 1) * CPC] = res[m]["right_out"]
    return out


# revision 9
# speedup vs baseline: 1.0580x; 1.0133x over previous
"""Cost-volume kernel for Trainium2 (Bass/Tile), 8-core SPMD.

Problem: left/right features [B=2, C=32, H=128, W=256] f32.
Output [B, 2C=64, D=48, H, W] where for disparity d in [-8, 40):
  out[:, 0:C,  d+8, h, x] = left[:, :, h, x]   if 0 <= x-d < W else 0
  out[:, C:2C, d+8, h, x] = right[:, :, h, x-d] if 0 <= x-d < W else 0

Sharding: channels split 4-per-core (8 cores, identical program).
Each core builds the full disparity band for its 4 left + 4 right
channels. Pure data-movement kernel, bound by the HBM write rate of
the 96 MiB/core output.

Perf notes (from NTFF traces):
  - HWDGE (sync/scalar) DMA rings only engage 8 of the 16 SDMA
    engines; SWDGE (gpsimd) engages all 16. All big transfers go SWDGE.
  - Every store is a full-width DMA with contiguous 8 KiB/partition
    source rows (128 descriptors of 8 KiB), which sustains near line
    rate. Right-side shifted windows are materialized by DVE copies
    into contiguous staging buffers to keep descriptors at 8 KiB.
  - Zero padding is produced in SBUF (host-padded right image, SBUF
    memsets for left), never as thin strided DRAM writes.
  - The right input arrives host-padded so no SBUF memset gates the
    first staging copies; left-buffer prep is emitted lazily to keep
    the gpsimd DMA FIFO from head-of-line blocking at startup.
"""

import numpy as np

B, C, H, W = 2, 32, 128, 256
MIN_D, MAX_D = -8, 40
D = MAX_D - MIN_D  # 48
N_CORES = 8
CPC = C // N_CORES  # 4 channels of each image per core
BC = B * CPC  # 8 (b, c) pairs per core

PAD_L = 39  # covers max shift d=39 (offset = x - d + PAD_L >= 0)
PAD_R = 9   # covers min shift d=-8 (x - d <= 263 -> offset 302 < 304)
WP = PAD_L + W + PAD_R  # 304

HL = 8            # h rows held per partition
HH = H // HL      # 16
NPART = BC * HH   # 128 partitions: p = (b*CPC + c)*HH + h_hi

POS_BUFS = 4  # left work buffers for d >= 0 (buffer j: d = j, j+4, ... asc)
NEG_BUFS = 2  # left work buffers for d < 0 (buffer j: d = -(j+1), -(j+1)-2, ... desc)
STAGE_BUFS = 16  # right staging rotation depth (deep: keeps SDMA queues fed)

# store order for the left side: negatives interleaved early; within a
# buffer positives ascend and negatives descend (zero regions only grow).
LEFT_ORDER = [0, -1, 1, -2, 2, 3, -3, 4, 5, -4, 6, 7, -5, 8, 9, -6, 10,
              11, -7, 12, 13, -8] + list(range(14, MAX_D))
assert sorted(LEFT_ORDER) == list(range(MIN_D, MAX_D))

_CACHE = {}


def _build_nc():
    import concourse.bacc as bacc
    import concourse.tile as tile
    import concourse.mybir as mybir

    f32 = mybir.dt.float32
    nc = bacc.Bacc(
        "TRN2",
        target_bir_lowering=False,
        debug=False,
        enable_asserts=False,
        num_devices=N_CORES,
    )
    left_in = nc.dram_tensor("left_in", [B, CPC, H, W], f32, kind="ExternalInput")
    right_in = nc.dram_tensor(
        "right_in", [B, CPC, H, WP], f32, kind="ExternalInput"
    )  # host-padded with zeros: data columns at [PAD_L, PAD_L + W)
    left_out = nc.dram_tensor(
        "left_out", [B, CPC, D, H, W], f32, kind="ExternalOutput"
    )
    right_out = nc.dram_tensor(
        "right_out", [B, CPC, D, H, W], f32, kind="ExternalOutput"
    )

    with tile.TileContext(nc) as tc:
        with (
            tc.tile_pool(name="pool", bufs=1) as pool,
            tc.tile_pool(name="stpool", bufs=STAGE_BUFS) as stpool,
        ):
            # ---- right image (pre-padded), loaded once ----
            rp = pool.tile([NPART, HL * WP], f32, tag="rp")
            rp3 = rp[:].rearrange("p (h w) -> p h w", h=HL)
            nc.gpsimd.dma_start(rp[:], right_in.ap())

            # ---- left work buffers; pos[0] is the load target, the rest
            # are DVE-copied lazily on first use ----
            pos = []
            neg = []
            for j in range(POS_BUFS):
                t = pool.tile([NPART, HL * W], f32, tag=f"lp{j}")
                pos.append([t, t[:].rearrange("p (h w) -> p h w", h=HL), False])
            for j in range(NEG_BUFS):
                t = pool.tile([NPART, HL * W], f32, tag=f"ln{j}")
                neg.append([t, t[:].rearrange("p (h w) -> p h w", h=HL), False])
            pos[0][2] = True  # loaded directly, no copy needed
            nc.gpsimd.dma_start(pos[0][0][:], left_in.ap())

            def emit_left(d):
                if d >= 0:
                    buf = pos[d % POS_BUFS]
                    t, t3, ready = buf
                    if not ready:
                        nc.scalar.copy(t[:], pos[0][0][:])
                        if d > 0:
                            nc.vector.memset(t3[:, :, 0:d], 0.0)
                        buf[2] = True
                    elif d >= POS_BUFS:
                        nc.vector.memset(t3[:, :, d - POS_BUFS : d], 0.0)
                else:
                    buf = neg[(-d - 1) % NEG_BUFS]
                    t, t3, ready = buf
                    if not ready:
                        nc.scalar.copy(t[:], pos[0][0][:])
                        nc.vector.memset(t3[:, :, W + d : W], 0.0)
                        buf[2] = True
                    else:
                        nc.vector.memset(t3[:, :, W + d : W + d + NEG_BUFS], 0.0)
                nc.gpsimd.dma_start(left_out.ap()[:, :, d - MIN_D, :, :], t[:])

            def emit_right(di):
                d = di + MIN_D
                a = PAD_L - d
                stage = stpool.tile([NPART, HL * W], f32, tag="st")
                st3 = stage[:].rearrange("p (h w) -> p h w", h=HL)
                nc.vector.tensor_copy(st3[:], rp3[:, :, a : a + W])
                nc.gpsimd.dma_start(right_out.ap()[:, :, di, :, :], stage[:])

            for step in range(D):
                emit_right(step)
                emit_left(LEFT_ORDER[step])

    nc.compile()
    return nc


def _get_nc():
    if "nc" not in _CACHE:
        _CACHE["nc"] = _build_nc()
    return _CACHE["nc"]


def kernel(left_feat, right_feat):
    from concourse.bass_utils import run_bass_kernel_spmd

    left = np.ascontiguousarray(np.asarray(left_feat), dtype=np.float32)
    right = np.ascontiguousarray(np.asarray(right_feat), dtype=np.float32)
    assert left.shape == (B, C, H, W) and right.shape == (B, C, H, W)

    nc = _get_nc()
    right_pad = np.zeros((B, C, H, WP), dtype=np.float32)
    right_pad[:, :, :, PAD_L : PAD_L + W] = right
    in_maps = []
    for m in range(N_CORES):
        sl = slice(m * CPC, (m + 1) * CPC)
        in_maps.append(
            {
                "left_in": np.ascontiguousarray(left[:, sl]),
                "right_in": np.ascontiguousarray(right_pad[:, sl]),
            }
        )
    res = run_bass_kernel_spmd(nc, in_maps, core_ids=list(range(N_CORES))).results

    out = np.empty((B, 2 * C, D, H, W), dtype=np.float32)
    for m in range(N_CORES):
        sl = slice(m * CPC, (m + 1) * CPC)
        out[:, sl] = res[m]["left_out"]
        out[:, C + m * CPC : C + (m + 1) * CPC] = res[m]["right_out"]
    return out
